# revision 1
# baseline (speedup 1.0000x reference)
"""BiAttention Trainium2 kernel (8 NeuronCores, batch-parallel).

Problem (per batch element b, 8 of them -> one per core):
    A_proj = A @ W_A + b_A            [2048, 64]
    B_proj = B @ W_B + b_B            [2048, 64]
    S      = A_proj @ B_proj^T        [2048, 2048]
    A_star = softmax(S, axis=-1) @ B  [2048, 768]
    B_star = softmax(S, axis=0)^T @ A [2048, 768]

Key algebra used on-device (S is small: |S| < ~30, so exp(S) is safe in
f32/bf16 without max-subtraction):
    E = exp(S)
    A_star = diag(1/rowsum(E)) . (E @ B)
    B_star = diag(1/colsum(E)) . (E^T @ A)
rowsum/colsum are obtained for free by augmenting the moving operands
with a ones-column (E @ [B | 1] gives the row sums in the last column).

E is never materialized in full: score panels are recomputed per
512-wide output stripe (K=64 contraction - cheap) directly from the
projections, exp'd into bf16 packs, and immediately consumed as the
stationary operand of the big matmuls.
"""

import sys

if "/opt/trn_rl_repo" not in sys.path:
    sys.path.insert(0, "/opt/trn_rl_repo")

import numpy as np

import concourse.bass as bass
import concourse.mybir as mybir
import concourse.tile as tile
from concourse import bacc
from concourse.bass import ts
from concourse.bass_utils import run_bass_kernel_spmd

F32 = mybir.dt.float32
BF16 = mybir.dt.bfloat16
AF = mybir.ActivationFunctionType

L = 2048          # sequence length (both La and Lb)
D = 768           # model dim
H = 64            # projection dim
NT = L // 128     # 16 row/col tiles of 128
KD = D // 128     # 6 contraction tiles for the projections
NSUP = L // 512   # 4 supers (512-wide output stripes)
DP = D + 1        # moving operand width with the ones column

N_CORES = 8

_CACHE = {}


def _build():
    nc = bacc.Bacc("TRN2", target_bir_lowering=False, debug=False,
                   num_devices=N_CORES)
    A_d = nc.dram_tensor("A", [L, D], F32, kind="ExternalInput").ap()
    B_d = nc.dram_tensor("B", [L, D], F32, kind="ExternalInput").ap()
    WA_d = nc.dram_tensor("W_A", [D, H], F32, kind="ExternalInput").ap()
    WB_d = nc.dram_tensor("W_B", [D, H], F32, kind="ExternalInput").ap()
    bA_d = nc.dram_tensor("b_A", [H, 1], F32, kind="ExternalInput").ap()
    bB_d = nc.dram_tensor("b_B", [H, 1], F32, kind="ExternalInput").ap()
    AS_d = nc.dram_tensor("A_star", [L, D], F32, kind="ExternalOutput").ap()
    BS_d = nc.dram_tensor("B_star", [L, D], F32, kind="ExternalOutput").ap()

    with tile.TileContext(nc) as tc:
        with (
            tc.tile_pool(name="stage", bufs=3) as pstage,
            tc.tile_pool(name="mov", bufs=1) as pmov,
            tc.tile_pool(name="proj", bufs=1) as pproj,
            tc.tile_pool(name="pack", bufs=2) as ppack,
            tc.tile_pool(name="outp", bufs=4) as pout,
            tc.tile_pool(name="psum", bufs=2, space="PSUM") as pps,
        ):
            # ---- weights + biases ----
            w_sb = {}
            b_sb = {}
            for side, (W_dram, b_dram) in (
                ("A", (WA_d, bA_d)), ("B", (WB_d, bB_d))
            ):
                wst = pstage.tile([128, KD, H], F32, tag="wstage")
                nc.gpsimd.dma_start(
                    out=wst, in_=W_dram.rearrange("(k p) h -> p k h", p=128)
                )
                wb = pmov.tile([128, KD, H], BF16, tag=f"w{side}",
                               name=f"w{side}b")
                nc.vector.tensor_copy(out=wb, in_=wst)
                bt = pmov.tile([H, 1], F32, tag=f"b{side}", name=f"b{side}sb")
                nc.gpsimd.dma_start(out=bt, in_=b_dram)
                w_sb[side] = wb
                b_sb[side] = bt

            # ---- load A/B, cast to bf16 (with ones column), transpose ----
            aug = {}
            projT = {}
            with tc.tile_pool(name="trans", bufs=1) as ptr:
                for side, M_dram in (("A", A_d), ("B", B_d)):
                    mt = ptr.tile([128, KD, L], BF16, tag=f"t{side}",
                                  name=f"{side}_T")
                    ag = pmov.tile([128, NT, DP], BF16, tag=f"aug{side}",
                                   name=f"{side}_aug")
                    for i in range(NT):
                        st = pstage.tile([128, D], F32, tag="stage",
                                         name=f"st{side}{i}")
                        nc.gpsimd.dma_start(out=st, in_=M_dram[ts(i, 128), :])
                        nc.vector.tensor_copy(out=ag[:, i, 0:D], in_=st)
                        for j in range(KD):
                            nc.sync.dma_start_transpose(
                                out=mt[:, j, ts(i, 128)],
                                in_=ag[:, i, ts(j, 128)],
                            )
                    nc.vector.memset(ag[:, :, D:DP], 1.0)
                    aug[side] = ag

                    # ---- projection: projT[h, s] = sum_d W[d,h] M^T[d,s] ----
                    pT = pproj.tile([H, L], BF16, tag=f"p{side}",
                                    name=f"{side}_projT")
                    for n in range(2):          # 1024-wide chunks
                        ps = pps.tile([128, 1024], F32, tag="spack",
                                      name=f"psproj{side}{n}")
                        for nn in range(2):     # 512-wide matmuls
                            lo = n * 1024 + nn * 512
                            for k in range(KD):
                                nc.tensor.matmul(
                                    ps[:H, ts(nn, 512)],
                                    w_sb[side][:, k, :],
                                    mt[:, k, lo:lo + 512],
                                    start=(k == 0), stop=(k == KD - 1),
                                )
                        nc.scalar.activation(
                            out=pT[:, ts(n, 1024)], in_=ps[:H, :],
                            func=AF.Identity, bias=b_sb[side], scale=1.0,
                        )
                    projT[side] = pT
            # ptr closed: A_T/B_T SBUF space released

            # ---- main: per 512-wide output stripe ----
            # dirn "A": produce A_star rows; panels are E'[t, s-stripe]
            #   (lhsT = B_projT tiles, rhs = A_projT stripe), moving = B_aug
            # dirn "B": produce B_star rows; panels are E[s, t-stripe]
            #   (lhsT = A_projT tiles, rhs = B_projT stripe), moving = A_aug
            for dirn, pT_l, pT_r, mv, out_d in (
                ("A", projT["B"], projT["A"], aug["B"], AS_d),
                ("B", projT["A"], projT["B"], aug["A"], BS_d),
            ):
                for u in range(NSUP):
                    pk = ppack.tile([128, NT * 512], BF16, tag="pack",
                                    name=f"pk{dirn}{u}")
                    for jp in range(NT // 2):
                        ps = pps.tile([128, 1024], F32, tag="spack",
                                      name=f"pss{dirn}{u}{jp}")
                        for h2 in range(2):
                            j = jp * 2 + h2
                            nc.tensor.matmul(
                                ps[:, ts(h2, 512)],
                                pT_l[:, ts(j, 128)],
                                pT_r[:, ts(u, 512)],
                                start=True, stop=True,
                            )
                        nc.scalar.activation(
                            out=pk[:, jp * 1024:(jp + 1) * 1024], in_=ps,
                            func=AF.Exp,
                        )
                    for ii in range(4):
                        pa = pps.tile([128, 1024], F32, tag="accum",
                                      name=f"pa{dirn}{u}{ii}")
                        for j in range(NT):
                            lhs = pk[:, j * 512 + ii * 128:
                                     j * 512 + ii * 128 + 128]
                            nc.tensor.matmul(
                                pa[:, 0:512], lhs, mv[:, j, 0:512],
                                start=(j == 0), stop=(j == NT - 1),
                            )
                            nc.tensor.matmul(
                                pa[:, 512:DP], lhs, mv[:, j, 512:DP],
                                start=(j == 0), stop=(j == NT - 1),
                            )
                        rinv = pout.tile([128, 1], F32, tag="rinv",
                                         name=f"ri{dirn}{u}{ii}")
                        nc.vector.reciprocal(out=rinv, in_=pa[:, D:DP])
                        ot = pout.tile([128, D], F32, tag="ot",
                                       name=f"ot{dirn}{u}{ii}")
                        nc.vector.tensor_scalar_mul(ot, pa[:, 0:D], rinv)
                        nc.sync.dma_start(
                            out=out_d[ts(u * 4 + ii, 128), :], in_=ot
                        )

    nc.compile()
    return nc


def _get_nc():
    if "nc" not in _CACHE:
        _CACHE["nc"] = _build()
    return _CACHE["nc"]


def _run(inputs, trace=False):
    nc = _get_nc()
    A = np.ascontiguousarray(np.asarray(inputs["A"], dtype=np.float32))
    B = np.ascontiguousarray(np.asarray(inputs["B"], dtype=np.float32))
    W_A = np.ascontiguousarray(np.asarray(inputs["W_A"], dtype=np.float32))
    W_B = np.ascontiguousarray(np.asarray(inputs["W_B"], dtype=np.float32))
    b_A = np.asarray(inputs["b_A"], dtype=np.float32).reshape(H, 1)
    b_B = np.asarray(inputs["b_B"], dtype=np.float32).reshape(H, 1)
    in_maps = [
        {
            "A": A[c], "B": B[c],
            "W_A": W_A, "W_B": W_B,
            "b_A": b_A, "b_B": b_B,
        }
        for c in range(N_CORES)
    ]
    res = run_bass_kernel_spmd(nc, in_maps, list(range(N_CORES)), trace=trace)
    A_star = np.stack([res.results[c]["A_star"] for c in range(N_CORES)])
    B_star = np.stack([res.results[c]["B_star"] for c in range(N_CORES)])
    return A_star, B_star, res


def kernel(**inputs):
    A_star, B_star, _ = _run(inputs)
    return A_star, B_star


# revision 3
# speedup vs baseline: 1.9448x; 1.9448x over previous
"""BiAttention Trainium2 kernel (8 NeuronCores, batch-parallel).

Problem (per batch element b, 8 of them -> one per core):
    A_proj = A @ W_A + b_A            [2048, 64]
    B_proj = B @ W_B + b_B            [2048, 64]
    S      = A_proj @ B_proj^T        [2048, 2048]
    A_star = softmax(S, axis=-1) @ B  [2048, 768]
    B_star = softmax(S, axis=0)^T @ A [2048, 768]

Key algebra used on-device (S is small: |S| < ~30, so exp(S) is safe in
f32/bf16 without max-subtraction):
    E = exp(S)
    A_star = diag(1/rowsum(E)) . (E @ B)
    B_star = diag(1/colsum(E)) . (E^T @ A)
rowsum/colsum are obtained for free by augmenting the moving operands
with a ones-column (E @ [B | 1] gives the row sums in the last column).

E is never materialized in full: score panels are recomputed per
512-wide output stripe (K=64 contraction - cheap) directly from the
projections, exp'd into bf16 packs, and immediately consumed as the
stationary operand of the big matmuls.
"""

import sys

if "/opt/trn_rl_repo" not in sys.path:
    sys.path.insert(0, "/opt/trn_rl_repo")

import numpy as np

import concourse.bass as bass
import concourse.mybir as mybir
import concourse.tile as tile
from concourse import bacc
from concourse.bass import ts
from concourse.bass_utils import run_bass_kernel_spmd

F32 = mybir.dt.float32
BF16 = mybir.dt.bfloat16
AF = mybir.ActivationFunctionType

L = 2048          # sequence length (both La and Lb)
D = 768           # model dim
H = 64            # projection dim
NT = L // 128     # 16 row/col tiles of 128
KD = D // 128     # 6 contraction tiles for the projections
NSUP = L // 512   # 4 supers (512-wide output stripes)
DP = D + 1        # moving operand width with the ones column

N_CORES = 8

_CACHE = {}


def _build():
    nc = bacc.Bacc("TRN2", target_bir_lowering=False, debug=False,
                   num_devices=N_CORES)
    A_d = nc.dram_tensor("A", [L, D], F32, kind="ExternalInput").ap()
    B_d = nc.dram_tensor("B", [L, D], F32, kind="ExternalInput").ap()
    WA_d = nc.dram_tensor("W_A", [D, H], F32, kind="ExternalInput").ap()
    WB_d = nc.dram_tensor("W_B", [D, H], F32, kind="ExternalInput").ap()
    bA_d = nc.dram_tensor("b_A", [H, 1], F32, kind="ExternalInput").ap()
    bB_d = nc.dram_tensor("b_B", [H, 1], F32, kind="ExternalInput").ap()
    AS_d = nc.dram_tensor("A_star", [L, D], F32, kind="ExternalOutput").ap()
    BS_d = nc.dram_tensor("B_star", [L, D], F32, kind="ExternalOutput").ap()

    with tile.TileContext(nc) as tc:
        with (
            tc.tile_pool(name="stage", bufs=3) as pstage,
            tc.tile_pool(name="mov", bufs=1) as pmov,
            tc.tile_pool(name="proj", bufs=1) as pproj,
            tc.tile_pool(name="pack", bufs=2) as ppack,
            tc.tile_pool(name="outp", bufs=4) as pout,
            tc.tile_pool(name="psum", bufs=2, space="PSUM") as pps,
        ):
            # ---- weights + biases ----
            w_sb = {}
            b_sb = {}
            for side, (W_dram, b_dram) in (
                ("A", (WA_d, bA_d)), ("B", (WB_d, bB_d))
            ):
                wst = pstage.tile([128, KD, H], F32, tag="wstage")
                nc.gpsimd.dma_start(
                    out=wst, in_=W_dram.rearrange("(k p) h -> p k h", p=128)
                )
                wb = pmov.tile([128, KD, H], BF16, tag=f"w{side}",
                               name=f"w{side}b")
                nc.vector.tensor_copy(out=wb, in_=wst)
                bt = pmov.tile([H, 1], F32, tag=f"b{side}", name=f"b{side}sb")
                nc.gpsimd.dma_start(out=bt, in_=b_dram)
                w_sb[side] = wb
                b_sb[side] = bt

            # ---- load A/B, cast to bf16 (with ones column), transpose ----
            aug = {}
            projT = {}
            with tc.tile_pool(name="trans", bufs=1) as ptr:
                for side, M_dram in (("A", A_d), ("B", B_d)):
                    # blocked transpose target: mt[d_lo, i*KD+j, s_lo]
                    mt = ptr.tile([128, NT * KD, 128], BF16, tag="t",
                                  name=f"{side}_T")
                    # contiguous bf16 copy (transpose source)
                    pkd = ptr.tile([128, NT * D], BF16, tag="c",
                                   name=f"{side}_pkd")
                    ag = pmov.tile([128, NT, DP], BF16, tag=f"aug{side}",
                                   name=f"{side}_aug")
                    for i in range(NT):
                        st = pstage.tile([128, D], F32, tag="stage",
                                         name=f"st{side}{i}")
                        nc.gpsimd.dma_start(out=st, in_=M_dram[ts(i, 128), :])
                        nc.vector.tensor_copy(out=pkd[:, ts(i, D)], in_=st)
                        nc.vector.tensor_copy(out=ag[:, i, 0:D],
                                              in_=pkd[:, ts(i, D)])
                    nc.vector.memset(ag[:, :, D:DP], 1.0)
                    # one big blocked transpose per matrix
                    nc.sync.dma_start_transpose(out=mt, in_=pkd)
                    aug[side] = ag

                    # ---- projection: projT[h, s] = sum_d W[d,h] M^T[d,s] ----
                    # mt viewed as [p, i, j, q]: block b = i*KD + j
                    mtv = mt.rearrange("p (i j) q -> p i j q", j=KD)
                    pT = pproj.tile([H, L], BF16, tag=f"p{side}",
                                    name=f"{side}_projT")
                    for n in range(2):          # 1024-wide chunks
                        ps = pps.tile([128, 1024], F32, tag="spack",
                                      name=f"psproj{side}{n}")
                        for nn in range(2):     # 512-wide matmuls
                            i0 = n * 8 + nn * 4
                            for k in range(KD):
                                nc.tensor.matmul(
                                    ps[:H, ts(nn, 512)],
                                    w_sb[side][:, k, :],
                                    mtv[:, i0:i0 + 4, k, :],
                                    start=(k == 0), stop=(k == KD - 1),
                                )
                        nc.scalar.activation(
                            out=pT[:, ts(n, 1024)], in_=ps[:H, :],
                            func=AF.Identity, bias=b_sb[side], scale=1.0,
                        )
                    projT[side] = pT
            # ptr closed: A_T/B_T SBUF space released

            # ---- main: per 512-wide output stripe ----
            # dirn "A": produce A_star rows; panels are E'[t, s-stripe]
            #   (lhsT = B_projT tiles, rhs = A_projT stripe), moving = B_aug
            # dirn "B": produce B_star rows; panels are E[s, t-stripe]
            #   (lhsT = A_projT tiles, rhs = B_projT stripe), moving = A_aug
            for dirn, pT_l, pT_r, mv, out_d in (
                ("A", projT["B"], projT["A"], aug["B"], AS_d),
                ("B", projT["A"], projT["B"], aug["A"], BS_d),
            ):
                for u in range(NSUP):
                    pk = ppack.tile([128, NT * 512], BF16, tag="pack",
                                    name=f"pk{dirn}{u}")
                    for jp in range(NT // 2):
                        ps = pps.tile([128, 1024], F32, tag="spack",
                                      name=f"pss{dirn}{u}{jp}")
                        for h2 in range(2):
                            j = jp * 2 + h2
                            nc.tensor.matmul(
                                ps[:, ts(h2, 512)],
                                pT_l[:, ts(j, 128)],
                                pT_r[:, ts(u, 512)],
                                start=True, stop=True,
                            )
                        nc.scalar.activation(
                            out=pk[:, jp * 1024:(jp + 1) * 1024], in_=ps,
                            func=AF.Exp,
                        )
                    for ii in range(4):
                        pa = pps.tile([128, 1024], F32, tag="accum",
                                      name=f"pa{dirn}{u}{ii}")
                        for j in range(NT):
                            lhs = pk[:, j * 512 + ii * 128:
                                     j * 512 + ii * 128 + 128]
                            nc.tensor.matmul(
                                pa[:, 0:512], lhs, mv[:, j, 0:512],
                                start=(j == 0), stop=(j == NT - 1),
                            )
                            nc.tensor.matmul(
                                pa[:, 512:DP], lhs, mv[:, j, 512:DP],
                                start=(j == 0), stop=(j == NT - 1),
                            )
                        rinv = pout.tile([128, 1], F32, tag="rinv",
                                         name=f"ri{dirn}{u}{ii}")
                        nc.vector.reciprocal(out=rinv, in_=pa[:, D:DP])
                        ot = pout.tile([128, D], F32, tag="ot",
                                       name=f"ot{dirn}{u}{ii}")
                        nc.vector.tensor_scalar_mul(ot, pa[:, 0:D], rinv)
                        nc.sync.dma_start(
                            out=out_d[ts(u * 4 + ii, 128), :], in_=ot
                        )

    nc.compile()
    return nc


def _get_nc():
    if "nc" not in _CACHE:
        _CACHE["nc"] = _build()
    return _CACHE["nc"]


def _run(inputs, trace=False):
    nc = _get_nc()
    A = np.ascontiguousarray(np.asarray(inputs["A"], dtype=np.float32))
    B = np.ascontiguousarray(np.asarray(inputs["B"], dtype=np.float32))
    W_A = np.ascontiguousarray(np.asarray(inputs["W_A"], dtype=np.float32))
    W_B = np.ascontiguousarray(np.asarray(inputs["W_B"], dtype=np.float32))
    b_A = np.asarray(inputs["b_A"], dtype=np.float32).reshape(H, 1)
    b_B = np.asarray(inputs["b_B"], dtype=np.float32).reshape(H, 1)
    in_maps = [
        {
            "A": A[c], "B": B[c],
            "W_A": W_A, "W_B": W_B,
            "b_A": b_A, "b_B": b_B,
        }
        for c in range(N_CORES)
    ]
    res = run_bass_kernel_spmd(nc, in_maps, list(range(N_CORES)), trace=trace)
    A_star = np.stack([res.results[c]["A_star"] for c in range(N_CORES)])
    B_star = np.stack([res.results[c]["B_star"] for c in range(N_CORES)])
    return A_star, B_star, res


def kernel(**inputs):
    A_star, B_star, _ = _run(inputs)
    return A_star, B_star
